# revision 1
# baseline (speedup 1.0000x reference)
"""Trainium2 Bass kernel for an AttentionBlock (GroupNorm + 4-head self-attention + proj).

Sharding: 8 cores = 4 batches x 2 head-pairs. Core c handles batch c//2, heads
{2j, 2j+1} where j = c%2. Each core: groupnorm of x[b] (duplicated across the
pair of cores), QKV for its 128 feature channels, transposed-score flash
attention (no max subtraction -- scores are ~N(0,1)), partial projection.
Host sums the two partial projections per batch and adds residual + proj bias.

Layout convention on device: features/keys on partitions, tokens on free dim.
  Q, K: (128=2x64 head dims, 4096 tokens)
  VT:   token-major tiles (128 tokens, [V_A(64)|1|V_B(64)|1]) -> M=65 matmuls
        compute attention output AND softmax denominator in one pass.
All matmuls run as float32r (TF32-like, full rate at N>=512).
"""
import sys

sys.path.insert(0, "/opt/trn_rl_repo")

import numpy as np

import concourse.bacc as bacc
import concourse.mybir as mybir
import concourse.tile as tile
from concourse import bass_utils, library_config

F32 = mybir.dt.float32
F32R = mybir.dt.float32r
BF16 = mybir.dt.bfloat16
AF = mybir.ActivationFunctionType
ALU = mybir.AluOpType
AX = mybir.AxisListType

B, C, H, W = 4, 256, 64, 64
N = H * W                  # 4096 tokens
NG = 8                     # groupnorm groups
GSZ = C // NG              # 32 channels per group
NQC = 8                    # query chunks of 512
QC = 512
NKT = 32                   # key tiles of 128
KT = 128
INV_GN = 1.0 / GSZ         # stats are per-partition means
SCALE = 1.0 / 8.0          # hd^-0.5

_CACHE: dict = {}


def _build():
    nc = bacc.Bacc("TRN2", target_bir_lowering=False, debug=False,
                   enable_asserts=False)

    xb = nc.dram_tensor("xb", [8, 128, 1024], BF16, kind="ExternalInput")
    wsl = nc.dram_tensor("wsl", [2, 128, 386], F32, kind="ExternalInput")
    csts = nc.dram_tensor("csts", [128, 10], F32, kind="ExternalInput")
    bv = nc.dram_tensor("bv", [1, 130], F32, kind="ExternalInput")
    selt = nc.dram_tensor("selt", [4, 128], F32, kind="ExternalInput")
    wpt = nc.dram_tensor("wpt", [128, 256], F32, kind="ExternalInput")
    yp = nc.dram_tensor("yp", [16, 128, 512], F32, kind="ExternalOutput")

    with tile.TileContext(nc) as tc:
        with (
            tc.tile_pool(name="cst", bufs=1) as cst,
            tc.tile_pool(name="big", bufs=1) as big,
            tc.tile_pool(name="pp", bufs=6) as pp,
            tc.tile_pool(name="sm", bufs=2) as sm,
            tc.tile_pool(name="yy", bufs=4) as yy,
            tc.tile_pool(name="dr", bufs=4, space="DRAM") as dr,
            tc.tile_pool(name="ps", bufs=2, space="PSUM") as ps,
            tc.tile_pool(name="po", bufs=4, space="PSUM") as po,
        ):
            # ---- constants ----
            W0 = cst.tile([128, 386], F32R, tag="w0")
            W1 = cst.tile([128, 386], F32R, tag="w1")
            WP = cst.tile([128, 256], F32R, tag="wp")
            CST = cst.tile([128, 10], F32, tag="cst")
            BQK = CST[:, 0:2]
            GAM = CST[:, 2:4]
            BET = CST[:, 4:6]
            SEL = CST[:, 6:10]
            BV = cst.tile([1, 130], F32, tag="bv")
            BVB = cst.tile([128, 130], F32, tag="bvb")
            SELT = cst.tile([4, 128], F32, tag="selt")
            ONE = cst.tile([1, 128], F32, tag="one")
            EPS = cst.tile([128, 1], F32, tag="eps")
            WARM = cst.tile([1, 1], F32, tag="warm")
            nc.vector.memset(WARM[:], 0.0)
            nc.scalar.activation(WARM[:], WARM[:], AF.Exp)

            # ---- load x (chunked, stats via one-pass bn_stats) ----
            NCH = 8
            CH = N // NCH   # 512
            X = [big.tile([128, N], BF16, tag=f"x{cc}", name=f"X{cc}") for cc in range(2)]
            Hh = [big.tile([128, N], F32R, tag=f"h{cc}", name=f"H{cc}") for cc in range(2)]
            BNS = [cst.tile([128, NCH * 6], F32, tag=f"bns{cc}", name=f"BNS{cc}") for cc in range(2)]
            MV = [cst.tile([128, 2], F32, tag=f"mv{cc}", name=f"MV{cc}") for cc in range(2)]
            ST = [cst.tile([128, 2], F32, tag=f"st{cc}", name=f"ST{cc}") for cc in range(2)]
            GS = cst.tile([4, 4], F32, tag="gs")
            gs_ps = po.tile([4, 4], F32, tag="po")
            SX = cst.tile([128, 4], F32, tag="sx")
            SQ = cst.tile([128, 4], F32, tag="sq")
            for i in range(4):
                for cc in range(2):
                    dsl = slice(i * 1024, (i + 1) * 1024)
                    nc.sync.dma_start(X[cc][:, dsl], xb.ap()[cc * 4 + i])
                    for h in range(2):
                        j = 2 * i + h
                        sl = slice(j * CH, (j + 1) * CH)
                        if cc == 1 and j < 4:
                            # first-arriving half-1 slices on ACT (idle early);
                            # scratch into Hh[1] (overwritten later by real H)
                            nc.scalar.activation(
                                Hh[1][:, sl], X[1][:, sl], AF.Identity,
                                accum_out=SX[:, j:j + 1])
                            nc.scalar.activation(
                                Hh[1][:, sl], X[1][:, sl], AF.Square,
                                accum_out=SQ[:, j:j + 1])
                        else:
                            nc.vector.bn_stats(BNS[cc][:, 6 * j:6 * j + 6],
                                               X[cc][:, sl])
            # weights & consts (needed later than x)
            nc.vector.memset(EPS[:], 1e-5)
            nc.vector.memset(ONE[:], 1.0)
            nc.sync.dma_start(CST[:], csts.ap())
            nc.sync.dma_start(BV[:], bv.ap())
            nc.sync.dma_start(SELT[:], selt.ap())
            nc.sync.dma_start(W0[:], wsl.ap()[0].bitcast(F32R))
            nc.sync.dma_start(W1[:], wsl.ap()[1].bitcast(F32R))
            nc.sync.dma_start(WP[:], wpt.ap().bitcast(F32R))
            # broadcast v-bias to all 128 partitions via K=1 matmul
            bvb_ps = po.tile([128, 130], F32, tag="po")
            nc.tensor.matmul(bvb_ps[:], ONE[0:1, 0:128], BV[:], start=True, stop=True)
            nc.vector.tensor_copy(BVB[:], bvb_ps[:])
            for cc in range(2):
                if cc == 0:
                    nc.vector.bn_aggr(MV[0][:], BNS[0][:])
                    # ST = [mean_p, E[x^2]_p]
                    nc.vector.tensor_mul(ST[0][:, 1:2], MV[0][:, 0:1], MV[0][:, 0:1])
                    nc.vector.tensor_add(ST[0][:, 1:2], ST[0][:, 1:2], MV[0][:, 1:2])
                    nc.vector.tensor_copy(ST[0][:, 0:1], MV[0][:, 0:1])
                else:
                    # merge ACT sums (slices 0-3) with bn stats (slices 4-7)
                    nc.vector.bn_aggr(MV[1][:], BNS[1][:, 24:48])
                    sxs = cst.tile([128, 1], F32, tag="sxs")
                    sqs = cst.tile([128, 1], F32, tag="sqs")
                    nc.vector.reduce_sum(sxs[:], SX[:], axis=AX.X)
                    nc.vector.reduce_sum(sqs[:], SQ[:], axis=AX.X)
                    # mean_p = 0.5*mean_bn + sxs/4096
                    nc.vector.tensor_scalar_mul(ST[1][:, 0:1], MV[1][:, 0:1], 0.5)
                    nc.vector.tensor_scalar_mul(sxs[:], sxs[:], 1.0 / 4096.0)
                    nc.vector.tensor_add(ST[1][:, 0:1], ST[1][:, 0:1], sxs[:])
                    # E2_p = 0.5*(var_bn + mean_bn^2) + sqs/4096
                    nc.vector.tensor_mul(ST[1][:, 1:2], MV[1][:, 0:1], MV[1][:, 0:1])
                    nc.vector.tensor_add(ST[1][:, 1:2], ST[1][:, 1:2], MV[1][:, 1:2])
                    nc.vector.tensor_scalar_mul(ST[1][:, 1:2], ST[1][:, 1:2], 0.5)
                    nc.vector.tensor_scalar_mul(sqs[:], sqs[:], 1.0 / 4096.0)
                    nc.vector.tensor_add(ST[1][:, 1:2], ST[1][:, 1:2], sqs[:])
                nc.tensor.matmul(gs_ps[:, 2 * cc:2 * cc + 2], SEL,
                                 ST[cc][:], start=True, stop=True)
                nc.vector.tensor_copy(GS[:, 2 * cc:2 * cc + 2],
                                      gs_ps[:, 2 * cc:2 * cc + 2])

            # per-channel scale/shift: s = gamma/sqrt(var+eps), t = beta - mean*s
            gn_st = []
            for cc in range(2):
                pc_ps = po.tile([128, 2], F32, tag="po")
                nc.tensor.matmul(pc_ps[:], SELT[:], GS[:, 2 * cc:2 * cc + 2],
                                 start=True, stop=True)
                mean = cst.tile([128, 1], F32, tag=f"mean{cc}")
                var = cst.tile([128, 1], F32, tag=f"var{cc}")
                sd = cst.tile([128, 1], F32, tag=f"sd{cc}")
                s_t = cst.tile([128, 1], F32, tag=f"s{cc}")
                t_t = cst.tile([128, 1], F32, tag=f"t{cc}")
                nc.vector.tensor_scalar_mul(mean[:], pc_ps[:, 0:1], INV_GN)
                nc.vector.tensor_scalar_mul(var[:], pc_ps[:, 1:2], INV_GN)
                # var = E[x^2] - mean^2
                nc.vector.scalar_tensor_tensor(
                    out=sd[:], in0=mean[:], scalar=-1.0, in1=mean[:],
                    op0=ALU.mult, op1=ALU.mult)
                nc.vector.tensor_add(var[:], var[:], sd[:])
                nc.scalar.activation(sd[:], var[:], AF.Sqrt, bias=EPS[:])
                nc.vector.reciprocal(s_t[:], sd[:])
                nc.vector.tensor_mul(s_t[:], s_t[:], GAM[:, cc:cc + 1])
                nc.vector.scalar_tensor_tensor(
                    out=t_t[:], in0=mean[:], scalar=-1.0, in1=s_t[:],
                    op0=ALU.mult, op1=ALU.mult)
                nc.vector.tensor_add(t_t[:], t_t[:], BET[:, cc:cc + 1])
                gn_st.append((s_t, t_t))

            # h = x*s + t; half 0 on ACT (idle during prologue), half 1 on DVE
            for i in range(4):
                sl = slice(i * 1024, (i + 1) * 1024)
                s_t, t_t = gn_st[0]
                nc.scalar.activation(Hh[0][:, sl], X[0][:, sl], AF.Identity,
                                     bias=t_t[:], scale=s_t[:])
                s_t, t_t = gn_st[1]
                nc.vector.tensor_scalar(
                    out=Hh[1][:, sl], in0=X[1][:, sl], scalar1=s_t[:],
                    scalar2=t_t[:], op0=ALU.mult, op1=ALU.add)

            # ---- QKV (interleaved into qc 0 so the exp stream starts early) ----
            Q = big.tile([128, N], F32R, tag="q")
            K = big.tile([128, N], F32R, tag="k")
            VT = big.tile([128, NKT * 130], BF16, tag="vt")

            def emit_q_chunk(ch):
                tok = slice(ch * QC, (ch + 1) * QC)
                q_ps = po.tile([128, QC], F32, tag="po", name=f"q_ps{ch}")
                nc.tensor.matmul(q_ps[:], W0[:, 0:128], Hh[0][:, tok],
                                 start=True, stop=False)
                nc.tensor.matmul(q_ps[:], W1[:, 0:128], Hh[1][:, tok],
                                 start=False, stop=True)
                nc.vector.tensor_scalar(out=Q[:, tok], in0=q_ps[:],
                                        scalar1=BQK[:, 0:1], scalar2=None,
                                        op0=ALU.add)

            def emit_k_chunk(ch):
                tok = slice(ch * QC, (ch + 1) * QC)
                k_ps = po.tile([128, QC], F32, tag="po", name=f"k_ps{ch}")
                nc.tensor.matmul(k_ps[:], W0[:, 128:256], Hh[0][:, tok],
                                 start=True, stop=False)
                nc.tensor.matmul(k_ps[:], W1[:, 128:256], Hh[1][:, tok],
                                 start=False, stop=True)
                nc.vector.tensor_scalar(out=K[:, tok], in0=k_ps[:],
                                        scalar1=BQK[:, 1:2], scalar2=None,
                                        op0=ALU.add)

            def emit_vt_tile(kt):
                tok = slice(kt * KT, (kt + 1) * KT)
                vt_ps = po.tile([128, 130], F32, tag="po", name=f"vt_ps{kt}")
                nc.tensor.matmul(vt_ps[:], Hb[0][:, tok], WVB[0][:],
                                 start=True, stop=False)
                nc.tensor.matmul(vt_ps[:], Hb[1][:, tok], WVB[1][:],
                                 start=False, stop=True)
                nc.vector.scalar_tensor_tensor(
                    out=VT[:, kt * 130:(kt + 1) * 130], in0=vt_ps[:],
                    scalar=1.0, in1=BVB[:], op0=ALU.mult, op1=ALU.add)

            # bf16 twins for the V path (cheap VT production matmuls)
            WVB = [cst.tile([128, 130], BF16, tag=f"wvb{cc}", name=f"WVB{cc}") for cc in range(2)]
            nc.vector.tensor_copy(WVB[0][:], W0[:, 256:386])
            nc.vector.tensor_copy(WVB[1][:], W1[:, 256:386])
            Hb = [big.tile([128, N], BF16, tag=f"hb{cc}", name=f"Hb{cc}") for cc in range(2)]
            for i in range(4):
                sl = slice(i * 1024, (i + 1) * 1024)
                for cc in range(2):
                    s_t, t_t = gn_st[cc]
                    nc.vector.tensor_scalar(
                        out=Hb[cc][:, sl], in0=X[cc][:, sl], scalar1=s_t[:],
                        scalar2=t_t[:], op0=ALU.mult, op1=ALU.add)

            emit_q_chunk(0)
            emit_k_chunk(0)

            # ---- attention + projection ----
            pending = None
            pv_queue = []
            for qc in range(NQC):
                qs = slice(qc * QC, (qc + 1) * QC)
                O_A = po.tile([65, QC], F32, tag="po", name=f"O_A{qc}")
                O_B = po.tile([65, QC], F32, tag="po", name=f"O_B{qc}")
                for kt in range(NKT):
                    if qc == 0:
                        if kt % 4 == 2 and kt // 4 < 7:
                            emit_k_chunk(kt // 4 + 1)
                        emit_vt_tile(kt)
                    if kt == 3 and pending is not None:
                        pending()
                        pending = None
                    if kt == 16 and qc < NQC - 1:
                        emit_q_chunk(qc + 1)
                    ks = slice(kt * KT, (kt + 1) * KT)
                    s_ps = ps.tile([128, 1024], F32, tag="s", name=f"s{qc}_{kt}")
                    nc.tensor.matmul(s_ps[:, 0:512], K[0:64, ks],
                                     Q[0:64, qs], start=True, stop=True)
                    nc.tensor.matmul(s_ps[:, 512:1024], K[64:128, ks],
                                     Q[64:128, qs], start=True, stop=True)
                    if len(pv_queue) == 2:
                        pv_queue.pop(0)()
                    p_t = pp.tile([128, 1024], BF16, tag="p", name=f"p{qc}_{kt}")
                    nc.scalar.activation(p_t[:], s_ps[:], AF.Exp, scale=SCALE)

                    def _pv(kt=kt, p_t=p_t, O_A=O_A, O_B=O_B):
                        nc.tensor.matmul(O_A[:], VT[:, kt * 130:kt * 130 + 65],
                                         p_t[:, 0:512],
                                         start=(kt == 0), stop=(kt == NKT - 1))
                        nc.tensor.matmul(O_B[:], VT[:, kt * 130 + 65:kt * 130 + 130],
                                         p_t[:, 512:1024],
                                         start=(kt == 0), stop=(kt == NKT - 1))
                    pv_queue.append(_pv)

                def finish(qc=qc, qs=qs, O_A=O_A, O_B=O_B):
                    # normalize: attn = O / denom; recip broadcast via a DRAM
                    # bounce (0-stride source DMA) -- costs no compute engine.
                    # Last qc: PE K=1 matmul broadcast instead (shorter chain,
                    # the score-psum pool is idle by then).
                    bcs = sm.tile([128, QC], F32, tag="bcs", name=f"bcs{qc}")
                    rA = sm.tile([1, QC], F32, tag="ra", name=f"rA{qc}")
                    rB = sm.tile([1, QC], F32, tag="rb", name=f"rB{qc}")
                    nc.vector.reciprocal(rA[:], O_A[64:65, :])
                    nc.vector.reciprocal(rB[:], O_B[64:65, :])
                    if qc == NQC - 1:
                        # tail: fp32 K=1 PE broadcast (score psum idle by now)
                        bc_ps = ps.tile([128, 1024], F32, tag="s", name="bc_tail")
                        nc.tensor.matmul(bc_ps[0:64, 0:512], ONE[0:1, 0:64],
                                         rA[:], start=True, stop=True)
                        nc.tensor.matmul(bc_ps[64:128, 0:512], ONE[0:1, 0:64],
                                         rB[:], start=True, stop=True)
                        nc.vector.tensor_copy(bcs[:], bc_ps[:, 0:512])
                    else:
                        rAd = dr.tile([1, QC], F32, tag="rad", name=f"rAd{qc}")
                        rBd = dr.tile([1, QC], F32, tag="rbd", name=f"rBd{qc}")
                        nc.sync.dma_start(rAd[:], rA[:])
                        nc.sync.dma_start(rBd[:], rB[:])
                        nc.sync.dma_start(bcs[0:64, :], rAd[:].broadcast_to((64, QC)))
                        nc.sync.dma_start(bcs[64:128, :], rBd[:].broadcast_to((64, QC)))
                    attn = sm.tile([128, QC], F32R, tag="attn", name=f"attn{qc}")
                    nc.vector.tensor_mul(attn[0:64, :], O_A[0:64, :], bcs[0:64, :])
                    nc.vector.tensor_mul(attn[64:128, :], O_B[0:64, :], bcs[64:128, :])
                    for half in range(2):
                        y_ps = po.tile([128, QC], F32, tag="po", name=f"y_ps{qc}_{half}")
                        nc.tensor.matmul(y_ps[:], WP[:, half * 128:(half + 1) * 128],
                                         attn[:], start=True, stop=True)
                        y_sb = yy.tile([128, QC], F32, tag="y", name=f"y_sb{qc}_{half}")
                        nc.vector.tensor_copy(y_sb[:], y_ps[:])
                        nc.sync.dma_start(yp.ap()[half * 8 + qc], y_sb[:])

                pending = finish
            for f in pv_queue:
                f()
            if pending is not None:
                pending()

    nc.compile()
    return nc


def _get_nc():
    if "nc" not in _CACHE:
        _CACHE["nc"] = _build()
    return _CACHE["nc"]


def build_in_maps(x, gn_gamma, gn_beta, w_qkv, b_qkv, w_proj):
    sel_np = np.zeros((128, 4), np.float32)
    for c in range(128):
        sel_np[c, c // 32] = 1.0
    selt_np = sel_np.T.copy()
    gmt_np = np.stack([gn_gamma[0:128], gn_gamma[128:256]], axis=1)
    btt_np = np.stack([gn_beta[0:128], gn_beta[128:256]], axis=1)

    in_maps = []
    for core in range(8):
        b, j = core // 2, core % 2
        r0 = 128 * j
        wsl_np = np.zeros((2, 128, 386), np.float32)
        for cc in range(2):
            cols = slice(cc * 128, (cc + 1) * 128)
            wsl_np[cc, :, 0:128] = w_qkv[r0:r0 + 128, cols].T
            wsl_np[cc, :, 128:256] = w_qkv[256 + r0:256 + r0 + 128, cols].T
            wsl_np[cc, :, 256:320] = w_qkv[512 + r0:512 + r0 + 64, cols].T
            wsl_np[cc, :, 321:385] = w_qkv[512 + r0 + 64:512 + r0 + 128, cols].T
        bqk_np = np.stack([b_qkv[r0:r0 + 128], b_qkv[256 + r0:256 + r0 + 128]],
                          axis=1)
        bv_np = np.zeros((1, 130), np.float32)
        bv_np[0, 0:64] = b_qkv[512 + r0:512 + r0 + 64]
        bv_np[0, 64] = 1.0
        bv_np[0, 65:129] = b_qkv[512 + r0 + 64:512 + r0 + 128]
        bv_np[0, 129] = 1.0
        csts_np = np.concatenate([bqk_np, gmt_np, btt_np, sel_np], axis=1)
        import ml_dtypes
        xq = np.ascontiguousarray(
            x[b].reshape(2, 128, 4, 1024).transpose(0, 2, 1, 3)
            .reshape(8, 128, 1024).astype(ml_dtypes.bfloat16))
        in_maps.append({
            "xb": xq,
            "wsl": np.ascontiguousarray(wsl_np),
            "csts": np.ascontiguousarray(csts_np),
            "bv": np.ascontiguousarray(bv_np),
            "selt": selt_np,
            "wpt": np.ascontiguousarray(w_proj[:, r0:r0 + 128].T),
        })

    return in_maps


def kernel(x, gn_gamma, gn_beta, w_qkv, b_qkv, w_proj, b_proj, **_unused):
    x = np.ascontiguousarray(np.asarray(x, dtype=np.float32))
    gn_gamma = np.asarray(gn_gamma, dtype=np.float32)
    gn_beta = np.asarray(gn_beta, dtype=np.float32)
    w_qkv = np.asarray(w_qkv, dtype=np.float32)
    b_qkv = np.asarray(b_qkv, dtype=np.float32)
    w_proj = np.asarray(w_proj, dtype=np.float32)
    b_proj = np.asarray(b_proj, dtype=np.float32)

    nc = _get_nc()
    in_maps = build_in_maps(x, gn_gamma, gn_beta, w_qkv, b_qkv, w_proj)
    res = bass_utils.run_bass_kernel_spmd(nc, in_maps, core_ids=list(range(8)))
    _CACHE["last_result"] = res

    out = np.empty((B, C, N), np.float32)
    for b in range(B):
        ypsum = res.results[2 * b]["yp"] + res.results[2 * b + 1]["yp"]
        ypsum = ypsum.reshape(2, 8, 128, 512).transpose(0, 2, 1, 3).reshape(C, N)
        out[b] = ypsum + x[b].reshape(C, N) + b_proj[:, None]
    return out.reshape(B, C, H, W)



# revision 3
# speedup vs baseline: 1.0073x; 1.0073x over previous
"""Trainium2 Bass kernel for an AttentionBlock (GroupNorm + 4-head self-attention + proj).

Sharding: 8 cores = 4 batches x 2 head-pairs. Core c handles batch c//2, heads
{2j, 2j+1} where j = c%2. Each core: groupnorm of x[b] (duplicated across the
pair of cores), QKV for its 128 feature channels, transposed-score flash
attention (no max subtraction -- scores are ~N(0,1)), partial projection.
Host sums the two partial projections per batch and adds residual + proj bias.

Layout on device: features/keys on partitions, tokens on free dim.
  Q, K: bf16 (128 = 2x64 head dims, 4096 tokens)
  VT:   fp8-e5m2 token-major tiles (128 tokens, [V_A(64)|1|V_B(64)|1]) --
        attention output AND softmax denominator in one pass.
  probs: fp8-e5m2, written per 128-key tile by either
        - ACT: activation(Exp, scale=1/8) with e5m2 output, or
        - DVE: Schraudolph bit-trick: u8 = rne(s*log2(e)/2 + 60) IS the e5m2
          bit pattern of exp(s/8) (linear-interp exp2; sat-at-0 kills the
          negative tail). This splits the softmax stream across two engines.
  PV:   DoubleRow fp8 matmuls: one matmul consumes TWO key tiles (256-deep
        contraction) at 0.5 cycles/row -> 4x fewer PE cycles than bf16.
"""
import sys

sys.path.insert(0, "/opt/trn_rl_repo")

import numpy as np

import concourse.bacc as bacc
import concourse.mybir as mybir
import concourse.tile as tile
from concourse import bass_utils

F32 = mybir.dt.float32
F32R = mybir.dt.float32r
BF16 = mybir.dt.bfloat16
E5 = mybir.dt.float8e5
U8 = mybir.dt.uint8
AF = mybir.ActivationFunctionType
ALU = mybir.AluOpType
AX = mybir.AxisListType
DR = mybir.MatmulPerfMode.DoubleRow

B, C, H, W = 4, 256, 64, 64
N = H * W                  # 4096 tokens
NG = 8                     # groupnorm groups
GSZ = C // NG              # 32 channels per group
NQC = 8                    # query chunks of 512
QC = 512
NKT = 32                   # key tiles of 128
KT = 128
NPR = NKT // 2             # kt pairs
INV_GN = 1.0 / GSZ         # stats are per-partition means
SCALE = 1.0 / 8.0          # hd^-0.5
A_SCH = np.log2(np.e) / 8.0 * 4.0   # schraudolph mult (folds 1/8 score scale)
B_SCH = 60.0                        # e5m2 exponent bias 15 << 2

# exp engine schedule: number of DVE (schraudolph) tiles per qc, out of 32.
# qc 0 is DVE-light (DVE busy with K/VT movers there).
DVE_EXPS = [6, 14, 14, 14, 14, 14, 14, 14]

_CACHE: dict = {}


def _sched_dve(qc, kt):
    n = DVE_EXPS[qc]
    if n <= 0:
        return False
    step = 32.0 / n
    # spread n DVE tiles evenly over the 32 kt slots
    return int(kt // step) != int((kt - 1) // step) if kt > 0 else False


def _build():
    nc = bacc.Bacc("TRN2", target_bir_lowering=False, debug=False,
                   enable_asserts=False)

    xb = nc.dram_tensor("xb", [8, 128, 1024], BF16, kind="ExternalInput")
    wslb = nc.dram_tensor("wslb", [2, 128, 400], BF16, kind="ExternalInput")
    csts = nc.dram_tensor("csts", [128, 10], F32, kind="ExternalInput")
    bv16 = nc.dram_tensor("bv16", [1, 144], BF16, kind="ExternalInput")
    selt = nc.dram_tensor("selt", [4, 128], F32, kind="ExternalInput")
    wpt = nc.dram_tensor("wpt", [128, 256], BF16, kind="ExternalInput")
    yp = nc.dram_tensor("yp", [16, 128, 512], F32, kind="ExternalOutput")

    with tile.TileContext(nc) as tc:
        with (
            tc.tile_pool(name="cst", bufs=1) as cst,
            tc.tile_pool(name="big", bufs=1) as big,
            tc.tile_pool(name="pp", bufs=4) as pp,
            tc.tile_pool(name="sm", bufs=2) as sm,
            tc.tile_pool(name="yy", bufs=4) as yy,
            tc.tile_pool(name="dr", bufs=4, space="DRAM") as dr,
            tc.tile_pool(name="ps", bufs=2, space="PSUM") as ps,
            tc.tile_pool(name="po", bufs=4, space="PSUM") as po,
        ):
            # ---- constants ----
            W0 = cst.tile([128, 400], BF16, tag="w0")
            W1 = cst.tile([128, 400], BF16, tag="w1")
            WP = cst.tile([128, 256], BF16, tag="wp")
            CST = cst.tile([128, 10], F32, tag="cst")
            BQK = CST[:, 0:2]
            GAM = CST[:, 2:4]
            BET = CST[:, 4:6]
            SEL = CST[:, 6:10]
            BV = cst.tile([1, 144], BF16, tag="bv")
            SELT = cst.tile([4, 128], F32, tag="selt")
            ONE = cst.tile([1, 128], F32, tag="one")
            ONEB = cst.tile([1, 128], BF16, tag="oneb")
            EPS = cst.tile([128, 1], F32, tag="eps")

            # ---- load x (chunked, stats via one-pass bn_stats) ----
            NCH = 8
            CH = N // NCH   # 512
            X = [big.tile([128, N], BF16, tag=f"x{cc}", name=f"X{cc}") for cc in range(2)]
            Hb = [big.tile([128, N], BF16, tag=f"hb{cc}", name=f"Hb{cc}") for cc in range(2)]
            BNS = [cst.tile([128, NCH * 6], F32, tag=f"bns{cc}", name=f"BNS{cc}") for cc in range(2)]
            MV = [cst.tile([128, 2], F32, tag=f"mv{cc}", name=f"MV{cc}") for cc in range(2)]
            ST = [cst.tile([128, 2], F32, tag=f"st{cc}", name=f"ST{cc}") for cc in range(2)]
            GS = cst.tile([4, 4], F32, tag="gs")
            gs_ps = po.tile([4, 4], F32, tag="po")
            SX = cst.tile([128, 4], F32, tag="sx")
            SQ = cst.tile([128, 4], F32, tag="sq")
            for i in range(4):
                for cc in range(2):
                    dsl = slice(i * 1024, (i + 1) * 1024)
                    nc.sync.dma_start(X[cc][:, dsl], xb.ap()[cc * 4 + i])
                    for h in range(2):
                        j = 2 * i + h
                        sl = slice(j * CH, (j + 1) * CH)
                        if cc == 1 and j < 4:
                            # first-arriving half-1 slices on ACT (idle early);
                            # scratch into Hb[1] (overwritten later by real Hb)
                            nc.scalar.activation(
                                Hb[1][:, sl], X[1][:, sl], AF.Identity,
                                accum_out=SX[:, j:j + 1])
                            nc.scalar.activation(
                                Hb[1][:, sl], X[1][:, sl], AF.Square,
                                accum_out=SQ[:, j:j + 1])
                        else:
                            nc.vector.bn_stats(BNS[cc][:, 6 * j:6 * j + 6],
                                               X[cc][:, sl])
            # weights & consts (needed later than x)
            nc.vector.memset(EPS[:], 1e-5)
            nc.vector.memset(ONE[:], 1.0)
            nc.vector.memset(ONEB[:], 1.0)
            nc.sync.dma_start(CST[:], csts.ap())
            nc.sync.dma_start(BV[:], bv16.ap())
            nc.sync.dma_start(SELT[:], selt.ap())
            nc.sync.dma_start(W0[:], wslb.ap()[0])
            nc.sync.dma_start(W1[:], wslb.ap()[1])
            nc.sync.dma_start(WP[:], wpt.ap())
            for cc in range(2):
                if cc == 0:
                    nc.vector.bn_aggr(MV[0][:], BNS[0][:])
                    # ST = [mean_p, E[x^2]_p]
                    nc.vector.tensor_mul(ST[0][:, 1:2], MV[0][:, 0:1], MV[0][:, 0:1])
                    nc.vector.tensor_add(ST[0][:, 1:2], ST[0][:, 1:2], MV[0][:, 1:2])
                    nc.vector.tensor_copy(ST[0][:, 0:1], MV[0][:, 0:1])
                else:
                    # merge ACT sums (slices 0-3) with bn stats (slices 4-7)
                    nc.vector.bn_aggr(MV[1][:], BNS[1][:, 24:48])
                    sxs = cst.tile([128, 1], F32, tag="sxs")
                    sqs = cst.tile([128, 1], F32, tag="sqs")
                    nc.vector.reduce_sum(sxs[:], SX[:], axis=AX.X)
                    nc.vector.reduce_sum(sqs[:], SQ[:], axis=AX.X)
                    # mean_p = 0.5*mean_bn + sxs/4096
                    nc.vector.tensor_scalar_mul(ST[1][:, 0:1], MV[1][:, 0:1], 0.5)
                    nc.vector.tensor_scalar_mul(sxs[:], sxs[:], 1.0 / 4096.0)
                    nc.vector.tensor_add(ST[1][:, 0:1], ST[1][:, 0:1], sxs[:])
                    # E2_p = 0.5*(var_bn + mean_bn^2) + sqs/4096
                    nc.vector.tensor_mul(ST[1][:, 1:2], MV[1][:, 0:1], MV[1][:, 0:1])
                    nc.vector.tensor_add(ST[1][:, 1:2], ST[1][:, 1:2], MV[1][:, 1:2])
                    nc.vector.tensor_scalar_mul(ST[1][:, 1:2], ST[1][:, 1:2], 0.5)
                    nc.vector.tensor_scalar_mul(sqs[:], sqs[:], 1.0 / 4096.0)
                    nc.vector.tensor_add(ST[1][:, 1:2], ST[1][:, 1:2], sqs[:])
                nc.tensor.matmul(gs_ps[:, 2 * cc:2 * cc + 2], SEL,
                                 ST[cc][:], start=True, stop=True)
                nc.vector.tensor_copy(GS[:, 2 * cc:2 * cc + 2],
                                      gs_ps[:, 2 * cc:2 * cc + 2])

            # per-channel scale/shift: s = gamma/sqrt(var+eps), t = beta - mean*s
            gn_st = []
            for cc in range(2):
                pc_ps = po.tile([128, 2], F32, tag="po")
                nc.tensor.matmul(pc_ps[:], SELT[:], GS[:, 2 * cc:2 * cc + 2],
                                 start=True, stop=True)
                mean = cst.tile([128, 1], F32, tag=f"mean{cc}")
                var = cst.tile([128, 1], F32, tag=f"var{cc}")
                sd = cst.tile([128, 1], F32, tag=f"sd{cc}")
                s_t = cst.tile([128, 1], F32, tag=f"s{cc}")
                t_t = cst.tile([128, 1], F32, tag=f"t{cc}")
                nc.vector.tensor_scalar_mul(mean[:], pc_ps[:, 0:1], INV_GN)
                nc.vector.tensor_scalar_mul(var[:], pc_ps[:, 1:2], INV_GN)
                # var = E[x^2] - mean^2
                nc.vector.scalar_tensor_tensor(
                    out=sd[:], in0=mean[:], scalar=-1.0, in1=mean[:],
                    op0=ALU.mult, op1=ALU.mult)
                nc.vector.tensor_add(var[:], var[:], sd[:])
                nc.scalar.activation(sd[:], var[:], AF.Sqrt, bias=EPS[:])
                nc.vector.reciprocal(s_t[:], sd[:])
                nc.vector.tensor_mul(s_t[:], s_t[:], GAM[:, cc:cc + 1])
                nc.vector.scalar_tensor_tensor(
                    out=t_t[:], in0=mean[:], scalar=-1.0, in1=s_t[:],
                    op0=ALU.mult, op1=ALU.mult)
                nc.vector.tensor_add(t_t[:], t_t[:], BET[:, cc:cc + 1])
                gn_st.append((s_t, t_t))

            # h = x*s + t (bf16); half 0 on DVE (2x mode), half 1 on ACT/Pool
            for i in range(4):
                sl = slice(i * 1024, (i + 1) * 1024)
                s_t, t_t = gn_st[0]
                nc.vector.tensor_scalar(
                    out=Hb[0][:, sl], in0=X[0][:, sl], scalar1=s_t[:],
                    scalar2=t_t[:], op0=ALU.mult, op1=ALU.add)
                s_t, t_t = gn_st[1]
                nc.gpsimd.tensor_scalar(
                    out=Hb[1][:, sl], in0=X[1][:, sl], scalar1=s_t[:],
                    scalar2=t_t[:], op0=ALU.mult, op1=ALU.add)

            # ---- QKV ----
            Q = big.tile([128, N], BF16, tag="q")
            K = big.tile([128, N], BF16, tag="k")
            VT = big.tile([128, NKT * 144], E5, tag="vt")

            def emit_q_chunk(ch):
                tok = slice(ch * QC, (ch + 1) * QC)
                q_ps = po.tile([128, QC], F32, tag="po", name=f"q_ps{ch}")
                nc.tensor.matmul(q_ps[:], W0[:, 0:128], Hb[0][:, tok],
                                 start=True, stop=False)
                nc.tensor.matmul(q_ps[:], W1[:, 0:128], Hb[1][:, tok],
                                 start=False, stop=True)
                nc.vector.tensor_scalar(out=Q[:, tok], in0=q_ps[:],
                                        scalar1=BQK[:, 0:1], scalar2=None,
                                        op0=ALU.add)

            def emit_k_chunk(ch):
                tok = slice(ch * QC, (ch + 1) * QC)
                k_ps = po.tile([128, QC], F32, tag="po", name=f"k_ps{ch}")
                nc.tensor.matmul(k_ps[:], W0[:, 128:256], Hb[0][:, tok],
                                 start=True, stop=False)
                nc.tensor.matmul(k_ps[:], W1[:, 128:256], Hb[1][:, tok],
                                 start=False, stop=True)
                nc.vector.tensor_scalar(out=K[:, tok], in0=k_ps[:],
                                        scalar1=BQK[:, 1:2], scalar2=None,
                                        op0=ALU.add)

            def emit_vt_tile(kt):
                tok = slice(kt * KT, (kt + 1) * KT)
                vt_ps = po.tile([128, 144], F32, tag="po", name=f"vt_ps{kt}")
                nc.tensor.matmul(vt_ps[:], Hb[0][:, tok], W0[:, 256:400],
                                 start=True, stop=False)
                nc.tensor.matmul(vt_ps[:], Hb[1][:, tok], W1[:, 256:400],
                                 start=False, stop=False)
                # V bias (+ the denominator 1s column) via rank-1 accumulate
                nc.tensor.matmul(vt_ps[:], ONEB[0:1, :], BV[:],
                                 start=False, stop=True)
                nc.vector.tensor_copy(VT[:, kt * 144:(kt + 1) * 144], vt_ps[:])

            emit_q_chunk(0)
            emit_k_chunk(0)

            # ---- attention + projection ----
            pending = None
            pv_queue = []
            for qc in range(NQC):
                qs = slice(qc * QC, (qc + 1) * QC)
                O_A = po.tile([72, QC], F32, tag="po", name=f"O_A{qc}")
                O_B = po.tile([72, QC], F32, tag="po", name=f"O_B{qc}")
                for pr in range(NPR):
                    P8 = pp.tile([128, 2048], E5, tag="p", name=f"p{qc}_{pr}")
                    for sub in range(2):
                        kt = 2 * pr + sub
                        if qc == 0:
                            if kt % 4 == 2 and kt // 4 < 7:
                                emit_k_chunk(kt // 4 + 1)
                            emit_vt_tile(kt)
                        if pr == 1 and sub == 1 and pending is not None:
                            pending()
                            pending = None
                        if pr == 8 and sub == 0 and qc < NQC - 1:
                            emit_q_chunk(qc + 1)
                        ks = slice(kt * KT, (kt + 1) * KT)
                        s_ps = ps.tile([128, 1024], F32, tag="s",
                                       name=f"s{qc}_{kt}")
                        nc.tensor.matmul(s_ps[:, 0:512], K[0:64, ks],
                                         Q[0:64, qs], start=True, stop=True)
                        nc.tensor.matmul(s_ps[:, 512:1024], K[64:128, ks],
                                         Q[64:128, qs], start=True, stop=True)
                        if len(pv_queue) == 2:
                            pv_queue.pop(0)()
                        dst = P8[:, sub * 1024:(sub + 1) * 1024]
                        if _sched_dve(qc, kt):
                            nc.vector.tensor_scalar(
                                out=dst.bitcast(U8), in0=s_ps[:],
                                scalar1=A_SCH, scalar2=B_SCH,
                                op0=ALU.mult, op1=ALU.add)
                        else:
                            nc.scalar.activation(dst, s_ps[:], AF.Exp,
                                                 scale=SCALE)

                    def _pv(pr=pr, P8=P8, O_A=O_A, O_B=O_B):
                        vt_ap = VT[:].rearrange("p (t x) -> p t x", t=NKT)[
                            :, 2 * pr:2 * pr + 2, :]
                        p_ap = P8[:].rearrange("p (t x) -> p t x", t=2)
                        nc.tensor.matmul(O_A[:], vt_ap[:, :, 0:72],
                                         p_ap[:, :, 0:512],
                                         start=(pr == 0), stop=(pr == NPR - 1),
                                         perf_mode=DR)
                        nc.tensor.matmul(O_B[:], vt_ap[:, :, 72:144],
                                         p_ap[:, :, 512:1024],
                                         start=(pr == 0), stop=(pr == NPR - 1),
                                         perf_mode=DR)
                    pv_queue.append(_pv)

                def finish(qc=qc, qs=qs, O_A=O_A, O_B=O_B):
                    # normalize: attn = O / denom; recip broadcast via a DRAM
                    # bounce (0-stride source DMA) -- costs no compute engine.
                    # Last qc: PE K=1 matmul broadcast instead (shorter chain,
                    # the score-psum pool is idle by then).
                    bcs = sm.tile([128, QC], F32, tag="bcs", name=f"bcs{qc}")
                    rA = sm.tile([1, QC], F32, tag="ra", name=f"rA{qc}")
                    rB = sm.tile([1, QC], F32, tag="rb", name=f"rB{qc}")
                    nc.vector.reciprocal(rA[:], O_A[64:65, :])
                    nc.vector.reciprocal(rB[:], O_B[64:65, :])
                    if qc == NQC - 1:
                        # tail: fp32 K=1 PE broadcast (score psum idle by now)
                        bc_ps = ps.tile([128, 1024], F32, tag="s", name="bc_tail")
                        nc.tensor.matmul(bc_ps[0:64, 0:512], ONE[0:1, 0:64],
                                         rA[:], start=True, stop=True)
                        nc.tensor.matmul(bc_ps[64:128, 0:512], ONE[0:1, 0:64],
                                         rB[:], start=True, stop=True)
                        nc.vector.tensor_copy(bcs[:], bc_ps[:, 0:512])
                    else:
                        rAd = dr.tile([1, QC], F32, tag="rad", name=f"rAd{qc}")
                        rBd = dr.tile([1, QC], F32, tag="rbd", name=f"rBd{qc}")
                        nc.sync.dma_start(rAd[:], rA[:])
                        nc.sync.dma_start(rBd[:], rB[:])
                        nc.sync.dma_start(bcs[0:64, :], rAd[:].broadcast_to((64, QC)))
                        nc.sync.dma_start(bcs[64:128, :], rBd[:].broadcast_to((64, QC)))
                    attn = sm.tile([128, QC], BF16, tag="attn", name=f"attn{qc}")
                    nc.vector.tensor_mul(attn[0:64, :], O_A[0:64, :], bcs[0:64, :])
                    nc.vector.tensor_mul(attn[64:128, :], O_B[0:64, :], bcs[64:128, :])
                    for half in range(2):
                        y_ps = po.tile([128, QC], F32, tag="po", name=f"y_ps{qc}_{half}")
                        nc.tensor.matmul(y_ps[:], WP[:, half * 128:(half + 1) * 128],
                                         attn[:], start=True, stop=True)
                        y_sb = yy.tile([128, QC], F32, tag="y", name=f"y_sb{qc}_{half}")
                        nc.scalar.activation(y_sb[:], y_ps[:], AF.Copy)
                        nc.sync.dma_start(yp.ap()[half * 8 + qc], y_sb[:])

                pending = finish
            for f in pv_queue:
                f()
            if pending is not None:
                pending()

    nc.compile()
    return nc


def _get_nc():
    if "nc" not in _CACHE:
        _CACHE["nc"] = _build()
    return _CACHE["nc"]


def build_in_maps(x, gn_gamma, gn_beta, w_qkv, b_qkv, w_proj):
    import ml_dtypes
    sel_np = np.zeros((128, 4), np.float32)
    for c in range(128):
        sel_np[c, c // 32] = 1.0
    selt_np = sel_np.T.copy()
    gmt_np = np.stack([gn_gamma[0:128], gn_gamma[128:256]], axis=1)
    btt_np = np.stack([gn_beta[0:128], gn_beta[128:256]], axis=1)

    in_maps = []
    for core in range(8):
        b, j = core // 2, core % 2
        r0 = 128 * j
        wsl_np = np.zeros((2, 128, 400), np.float32)
        for cc in range(2):
            cols = slice(cc * 128, (cc + 1) * 128)
            wsl_np[cc, :, 0:128] = w_qkv[r0:r0 + 128, cols].T
            wsl_np[cc, :, 128:256] = w_qkv[256 + r0:256 + r0 + 128, cols].T
            wsl_np[cc, :, 256:320] = w_qkv[512 + r0:512 + r0 + 64, cols].T
            wsl_np[cc, :, 328:392] = w_qkv[512 + r0 + 64:512 + r0 + 128, cols].T
        bqk_np = np.stack([b_qkv[r0:r0 + 128], b_qkv[256 + r0:256 + r0 + 128]],
                          axis=1)
        bv_np = np.zeros((1, 144), np.float32)
        bv_np[0, 0:64] = b_qkv[512 + r0:512 + r0 + 64]
        bv_np[0, 64] = 1.0
        bv_np[0, 72:136] = b_qkv[512 + r0 + 64:512 + r0 + 128]
        bv_np[0, 136] = 1.0
        csts_np = np.concatenate([bqk_np, gmt_np, btt_np, sel_np], axis=1)
        xq = np.ascontiguousarray(
            x[b].reshape(2, 128, 4, 1024).transpose(0, 2, 1, 3)
            .reshape(8, 128, 1024).astype(ml_dtypes.bfloat16))
        in_maps.append({
            "xb": xq,
            "wslb": np.ascontiguousarray(wsl_np.astype(ml_dtypes.bfloat16)),
            "csts": np.ascontiguousarray(csts_np),
            "bv16": np.ascontiguousarray(bv_np.astype(ml_dtypes.bfloat16)),
            "selt": selt_np,
            "wpt": np.ascontiguousarray(
                w_proj[:, r0:r0 + 128].T.astype(ml_dtypes.bfloat16)),
        })

    return in_maps


def kernel(x, gn_gamma, gn_beta, w_qkv, b_qkv, w_proj, b_proj, **_unused):
    x = np.ascontiguousarray(np.asarray(x, dtype=np.float32))
    gn_gamma = np.asarray(gn_gamma, dtype=np.float32)
    gn_beta = np.asarray(gn_beta, dtype=np.float32)
    w_qkv = np.asarray(w_qkv, dtype=np.float32)
    b_qkv = np.asarray(b_qkv, dtype=np.float32)
    w_proj = np.asarray(w_proj, dtype=np.float32)
    b_proj = np.asarray(b_proj, dtype=np.float32)

    nc = _get_nc()
    in_maps = build_in_maps(x, gn_gamma, gn_beta, w_qkv, b_qkv, w_proj)
    res = bass_utils.run_bass_kernel_spmd(nc, in_maps, core_ids=list(range(8)))
    _CACHE["last_result"] = res

    out = np.empty((B, C, N), np.float32)
    for b in range(B):
        ypsum = res.results[2 * b]["yp"] + res.results[2 * b + 1]["yp"]
        ypsum = ypsum.reshape(2, 8, 128, 512).transpose(0, 2, 1, 3).reshape(C, N)
        out[b] = ypsum + x[b].reshape(C, N) + b_proj[:, None]
    return out.reshape(B, C, H, W)


# revision 4
# speedup vs baseline: 1.0267x; 1.0192x over previous
"""Trainium2 Bass kernel for an AttentionBlock (GroupNorm + 4-head self-attention + proj).

Sharding: 8 cores = 4 batches x 2 head-pairs. Core c handles batch c//2, heads
{2j, 2j+1} where j = c%2. Each core: groupnorm of x[b] (duplicated across the
pair of cores), QKV for its 128 feature channels, transposed-score flash
attention (no max subtraction -- scores are ~N(0,1)), partial projection.
Host sums the two partial projections per batch and adds residual + proj bias.

Layout on device: features/keys on partitions, tokens on free dim.
  Q, K: bf16 (128 = 2x64 head dims, 4096 tokens)
  VT:   fp8-e5m2 token-major tiles (128 tokens, [V_A(64)|1|V_B(64)|1]) --
        attention output AND softmax denominator in one pass.
  probs: fp8-e5m2, written per 128-key tile by either
        - ACT: activation(Exp, scale=1/8) with e5m2 output, or
        - DVE: Schraudolph bit-trick: u8 = rne(s*log2(e)/2 + 60) IS the e5m2
          bit pattern of exp(s/8) (linear-interp exp2; sat-at-0 kills the
          negative tail). This splits the softmax stream across two engines.
  PV:   DoubleRow fp8 matmuls: one matmul consumes TWO key tiles (256-deep
        contraction) at 0.5 cycles/row -> 4x fewer PE cycles than bf16.
"""
import sys

sys.path.insert(0, "/opt/trn_rl_repo")

import numpy as np

import concourse.bacc as bacc
import concourse.mybir as mybir
import concourse.tile as tile
from concourse import bass_utils

F32 = mybir.dt.float32
F32R = mybir.dt.float32r
BF16 = mybir.dt.bfloat16
E5 = mybir.dt.float8e5
U8 = mybir.dt.uint8
AF = mybir.ActivationFunctionType
ALU = mybir.AluOpType
AX = mybir.AxisListType
DR = mybir.MatmulPerfMode.DoubleRow

B, C, H, W = 4, 256, 64, 64
N = H * W                  # 4096 tokens
NG = 8                     # groupnorm groups
GSZ = C // NG              # 32 channels per group
NQC = 8                    # query chunks of 512
QC = 512
NKT = 32                   # key tiles of 128
KT = 128
NPR = NKT // 2             # kt pairs
INV_GN = 1.0 / GSZ         # stats are per-partition means
SCALE = 1.0 / 8.0          # hd^-0.5
A_SCH = np.log2(np.e) / 8.0 * 4.0   # schraudolph mult (folds 1/8 score scale)
B_SCH = 60.0                        # e5m2 exponent bias 15 << 2

# exp engine schedule: number of DVE (schraudolph) tiles per qc, out of 32.
# qc 0 is DVE-light (DVE busy with K/VT movers there).
DVE_EXPS = [6, 14, 14, 14, 14, 14, 14, 14]

_CACHE: dict = {}


def _sched_dve(qc, kt):
    n = DVE_EXPS[qc]
    if n <= 0:
        return False
    step = 32.0 / n
    # spread n DVE tiles evenly over the 32 kt slots
    return int(kt // step) != int((kt - 1) // step) if kt > 0 else False


def _build():
    nc = bacc.Bacc("TRN2", target_bir_lowering=False, debug=False,
                   enable_asserts=False)

    xb = nc.dram_tensor("xb", [8, 128, 1024], BF16, kind="ExternalInput")
    wslb = nc.dram_tensor("wslb", [2, 128, 400], BF16, kind="ExternalInput")
    csts = nc.dram_tensor("csts", [128, 10], F32, kind="ExternalInput")
    bv16 = nc.dram_tensor("bv16", [1, 144], BF16, kind="ExternalInput")
    selt = nc.dram_tensor("selt", [4, 128], F32, kind="ExternalInput")
    wpt = nc.dram_tensor("wpt", [128, 256], BF16, kind="ExternalInput")
    yp = nc.dram_tensor("yp", [16, 128, 512], F32, kind="ExternalOutput")

    with tile.TileContext(nc) as tc:
        with (
            tc.tile_pool(name="cst", bufs=1) as cst,
            tc.tile_pool(name="big", bufs=1) as big,
            tc.tile_pool(name="pp", bufs=4) as pp,
            tc.tile_pool(name="sm", bufs=2) as sm,
            tc.tile_pool(name="yy", bufs=4) as yy,
            tc.tile_pool(name="dr", bufs=4, space="DRAM") as dr,
            tc.tile_pool(name="ps", bufs=3, space="PSUM") as ps,
            tc.tile_pool(name="po", bufs=2, space="PSUM") as po,
        ):
            # ---- constants ----
            W0 = cst.tile([128, 400], BF16, tag="w0")
            W1 = cst.tile([128, 400], BF16, tag="w1")
            WP = cst.tile([128, 256], BF16, tag="wp")
            CST = cst.tile([128, 10], F32, tag="cst")
            BQK = CST[:, 0:2]
            GAM = CST[:, 2:4]
            BET = CST[:, 4:6]
            SEL = CST[:, 6:10]
            BV = cst.tile([1, 144], BF16, tag="bv")
            SELT = cst.tile([4, 128], F32, tag="selt")
            ONE = cst.tile([1, 128], F32, tag="one")
            ONEB = cst.tile([1, 128], BF16, tag="oneb")
            EPS = cst.tile([128, 1], F32, tag="eps")

            # ---- load x (chunked, stats via one-pass bn_stats) ----
            NCH = 8
            CH = N // NCH   # 512
            X = [big.tile([128, N], BF16, tag=f"x{cc}", name=f"X{cc}") for cc in range(2)]
            Hb = [big.tile([128, N], BF16, tag=f"hb{cc}", name=f"Hb{cc}") for cc in range(2)]
            BNS = [cst.tile([128, NCH * 6], F32, tag=f"bns{cc}", name=f"BNS{cc}") for cc in range(2)]
            MV = [cst.tile([128, 2], F32, tag=f"mv{cc}", name=f"MV{cc}") for cc in range(2)]
            ST = [cst.tile([128, 2], F32, tag=f"st{cc}", name=f"ST{cc}") for cc in range(2)]
            GS = cst.tile([4, 4], F32, tag="gs")
            gs_ps = po.tile([4, 4], F32, tag="po")
            SX = cst.tile([128, 4], F32, tag="sx")
            SQ = cst.tile([128, 4], F32, tag="sq")
            for i in range(4):
                for cc in range(2):
                    dsl = slice(i * 1024, (i + 1) * 1024)
                    nc.sync.dma_start(X[cc][:, dsl], xb.ap()[cc * 4 + i])
                    for h in range(2):
                        j = 2 * i + h
                        sl = slice(j * CH, (j + 1) * CH)
                        if cc == 1 and j < 4:
                            # first-arriving half-1 slices on ACT (idle early);
                            # scratch into Hb[1] (overwritten later by real Hb)
                            nc.scalar.activation(
                                Hb[1][:, sl], X[1][:, sl], AF.Identity,
                                accum_out=SX[:, j:j + 1])
                            nc.scalar.activation(
                                Hb[1][:, sl], X[1][:, sl], AF.Square,
                                accum_out=SQ[:, j:j + 1])
                        else:
                            nc.vector.bn_stats(BNS[cc][:, 6 * j:6 * j + 6],
                                               X[cc][:, sl])
            # weights & consts (needed later than x)
            nc.vector.memset(EPS[:], 1e-5)
            nc.vector.memset(ONE[:], 1.0)
            nc.vector.memset(ONEB[:], 1.0)
            nc.sync.dma_start(CST[:], csts.ap())
            nc.sync.dma_start(BV[:], bv16.ap())
            nc.sync.dma_start(SELT[:], selt.ap())
            nc.sync.dma_start(W0[:], wslb.ap()[0])
            nc.sync.dma_start(W1[:], wslb.ap()[1])
            nc.sync.dma_start(WP[:], wpt.ap())
            for cc in range(2):
                if cc == 0:
                    nc.vector.bn_aggr(MV[0][:], BNS[0][:])
                    # ST = [mean_p, E[x^2]_p]
                    nc.vector.tensor_mul(ST[0][:, 1:2], MV[0][:, 0:1], MV[0][:, 0:1])
                    nc.vector.tensor_add(ST[0][:, 1:2], ST[0][:, 1:2], MV[0][:, 1:2])
                    nc.vector.tensor_copy(ST[0][:, 0:1], MV[0][:, 0:1])
                else:
                    # merge ACT sums (slices 0-3) with bn stats (slices 4-7)
                    nc.vector.bn_aggr(MV[1][:], BNS[1][:, 24:48])
                    sxs = cst.tile([128, 1], F32, tag="sxs")
                    sqs = cst.tile([128, 1], F32, tag="sqs")
                    nc.vector.reduce_sum(sxs[:], SX[:], axis=AX.X)
                    nc.vector.reduce_sum(sqs[:], SQ[:], axis=AX.X)
                    # mean_p = 0.5*mean_bn + sxs/4096
                    nc.vector.tensor_scalar_mul(ST[1][:, 0:1], MV[1][:, 0:1], 0.5)
                    nc.vector.tensor_scalar_mul(sxs[:], sxs[:], 1.0 / 4096.0)
                    nc.vector.tensor_add(ST[1][:, 0:1], ST[1][:, 0:1], sxs[:])
                    # E2_p = 0.5*(var_bn + mean_bn^2) + sqs/4096
                    nc.vector.tensor_mul(ST[1][:, 1:2], MV[1][:, 0:1], MV[1][:, 0:1])
                    nc.vector.tensor_add(ST[1][:, 1:2], ST[1][:, 1:2], MV[1][:, 1:2])
                    nc.vector.tensor_scalar_mul(ST[1][:, 1:2], ST[1][:, 1:2], 0.5)
                    nc.vector.tensor_scalar_mul(sqs[:], sqs[:], 1.0 / 4096.0)
                    nc.vector.tensor_add(ST[1][:, 1:2], ST[1][:, 1:2], sqs[:])
                nc.tensor.matmul(gs_ps[:, 2 * cc:2 * cc + 2], SEL,
                                 ST[cc][:], start=True, stop=True)
                nc.vector.tensor_copy(GS[:, 2 * cc:2 * cc + 2],
                                      gs_ps[:, 2 * cc:2 * cc + 2])

            # per-channel scale/shift: s = gamma/sqrt(var+eps), t = beta - mean*s
            gn_st = []
            for cc in range(2):
                pc_ps = po.tile([128, 2], F32, tag="po")
                nc.tensor.matmul(pc_ps[:], SELT[:], GS[:, 2 * cc:2 * cc + 2],
                                 start=True, stop=True)
                mean = cst.tile([128, 1], F32, tag=f"mean{cc}")
                var = cst.tile([128, 1], F32, tag=f"var{cc}")
                sd = cst.tile([128, 1], F32, tag=f"sd{cc}")
                s_t = cst.tile([128, 1], F32, tag=f"s{cc}")
                t_t = cst.tile([128, 1], F32, tag=f"t{cc}")
                nc.vector.tensor_scalar_mul(mean[:], pc_ps[:, 0:1], INV_GN)
                nc.vector.tensor_scalar_mul(var[:], pc_ps[:, 1:2], INV_GN)
                # var = E[x^2] - mean^2
                nc.vector.scalar_tensor_tensor(
                    out=sd[:], in0=mean[:], scalar=-1.0, in1=mean[:],
                    op0=ALU.mult, op1=ALU.mult)
                nc.vector.tensor_add(var[:], var[:], sd[:])
                nc.scalar.activation(sd[:], var[:], AF.Sqrt, bias=EPS[:])
                nc.vector.reciprocal(s_t[:], sd[:])
                nc.vector.tensor_mul(s_t[:], s_t[:], GAM[:, cc:cc + 1])
                nc.vector.scalar_tensor_tensor(
                    out=t_t[:], in0=mean[:], scalar=-1.0, in1=s_t[:],
                    op0=ALU.mult, op1=ALU.mult)
                nc.vector.tensor_add(t_t[:], t_t[:], BET[:, cc:cc + 1])
                gn_st.append((s_t, t_t))

            # h = x*s + t (bf16); half 0 on DVE (2x mode), half 1 on ACT/Pool
            for i in range(4):
                sl = slice(i * 1024, (i + 1) * 1024)
                s_t, t_t = gn_st[0]
                nc.vector.tensor_scalar(
                    out=Hb[0][:, sl], in0=X[0][:, sl], scalar1=s_t[:],
                    scalar2=t_t[:], op0=ALU.mult, op1=ALU.add)
                s_t, t_t = gn_st[1]
                nc.gpsimd.tensor_scalar(
                    out=Hb[1][:, sl], in0=X[1][:, sl], scalar1=s_t[:],
                    scalar2=t_t[:], op0=ALU.mult, op1=ALU.add)

            # ---- QKV ----
            Q = big.tile([128, N], BF16, tag="q")
            K = big.tile([128, N], BF16, tag="k")
            VT = big.tile([128, NKT * 144], E5, tag="vt")

            def emit_q_chunk(ch):
                tok = slice(ch * QC, (ch + 1) * QC)
                q_ps = ps.tile([128, QC], F32, tag="s", name=f"q_ps{ch}")
                nc.tensor.matmul(q_ps[:], W0[:, 0:128], Hb[0][:, tok],
                                 start=True, stop=False)
                nc.tensor.matmul(q_ps[:], W1[:, 0:128], Hb[1][:, tok],
                                 start=False, stop=True)
                nc.vector.tensor_scalar(out=Q[:, tok], in0=q_ps[:],
                                        scalar1=BQK[:, 0:1], scalar2=None,
                                        op0=ALU.add)

            def emit_k_chunk(ch):
                tok = slice(ch * QC, (ch + 1) * QC)
                k_ps = ps.tile([128, QC], F32, tag="s", name=f"k_ps{ch}")
                nc.tensor.matmul(k_ps[:], W0[:, 128:256], Hb[0][:, tok],
                                 start=True, stop=False)
                nc.tensor.matmul(k_ps[:], W1[:, 128:256], Hb[1][:, tok],
                                 start=False, stop=True)
                nc.vector.tensor_scalar(out=K[:, tok], in0=k_ps[:],
                                        scalar1=BQK[:, 1:2], scalar2=None,
                                        op0=ALU.add)

            def emit_vt_tile(kt):
                tok = slice(kt * KT, (kt + 1) * KT)
                vt_ps = ps.tile([128, 144], F32, tag="s", name=f"vt_ps{kt}")
                nc.tensor.matmul(vt_ps[:], Hb[0][:, tok], W0[:, 256:400],
                                 start=True, stop=False)
                nc.tensor.matmul(vt_ps[:], Hb[1][:, tok], W1[:, 256:400],
                                 start=False, stop=False)
                # V bias (+ the denominator 1s column) via rank-1 accumulate
                nc.tensor.matmul(vt_ps[:], ONEB[0:1, :], BV[:],
                                 start=False, stop=True)
                nc.vector.tensor_copy(VT[:, kt * 144:(kt + 1) * 144], vt_ps[:])

            emit_q_chunk(0)
            emit_k_chunk(0)

            # ---- attention + projection ----
            pending = None
            pv_queue = []
            for qc in range(NQC):
                qs = slice(qc * QC, (qc + 1) * QC)
                O_A = po.tile([72, QC], F32, tag="po", name=f"O_A{qc}")
                O_B = po.tile([72, QC], F32, tag="po", name=f"O_B{qc}")
                for pr in range(NPR):
                    P8 = pp.tile([128, 2048], E5, tag="p", name=f"p{qc}_{pr}")
                    for sub in range(2):
                        kt = 2 * pr + sub
                        if qc == 0:
                            if kt % 4 == 2 and kt // 4 < 7:
                                emit_k_chunk(kt // 4 + 1)
                            emit_vt_tile(kt)
                        if pr == 1 and sub == 1 and pending is not None:
                            pending()
                            pending = None
                        if pr == 8 and sub == 0 and qc < NQC - 1:
                            emit_q_chunk(qc + 1)
                        ks = slice(kt * KT, (kt + 1) * KT)
                        s_ps = ps.tile([128, 1024], F32, tag="s",
                                       name=f"s{qc}_{kt}")
                        nc.tensor.matmul(s_ps[:, 0:512], K[0:64, ks],
                                         Q[0:64, qs], start=True, stop=True)
                        nc.tensor.matmul(s_ps[:, 512:1024], K[64:128, ks],
                                         Q[64:128, qs], start=True, stop=True)
                        if len(pv_queue) == 2:
                            pv_queue.pop(0)()
                        dst = P8[:, sub * 1024:(sub + 1) * 1024]
                        if _sched_dve(qc, kt):
                            nc.vector.tensor_scalar(
                                out=dst.bitcast(U8), in0=s_ps[:],
                                scalar1=A_SCH, scalar2=B_SCH,
                                op0=ALU.mult, op1=ALU.add)
                        else:
                            nc.scalar.activation(dst, s_ps[:], AF.Exp,
                                                 scale=SCALE)

                    def _pv(pr=pr, P8=P8, O_A=O_A, O_B=O_B):
                        vt_ap = VT[:].rearrange("p (t x) -> p t x", t=NKT)[
                            :, 2 * pr:2 * pr + 2, :]
                        p_ap = P8[:].rearrange("p (t x) -> p t x", t=2)
                        nc.tensor.matmul(O_A[:], vt_ap[:, :, 0:72],
                                         p_ap[:, :, 0:512],
                                         start=(pr == 0), stop=(pr == NPR - 1),
                                         perf_mode=DR)
                        nc.tensor.matmul(O_B[:], vt_ap[:, :, 72:144],
                                         p_ap[:, :, 512:1024],
                                         start=(pr == 0), stop=(pr == NPR - 1),
                                         perf_mode=DR)
                    pv_queue.append(_pv)

                def finish(qc=qc, qs=qs, O_A=O_A, O_B=O_B):
                    # normalize: attn = O / denom; recip broadcast via a DRAM
                    # bounce (0-stride source DMA) -- costs no compute engine.
                    # Last qc: PE K=1 matmul broadcast instead (shorter chain,
                    # the score-psum pool is idle by then).
                    bcs = sm.tile([128, QC], F32, tag="bcs", name=f"bcs{qc}")
                    rA = sm.tile([1, QC], F32, tag="ra", name=f"rA{qc}")
                    rB = sm.tile([1, QC], F32, tag="rb", name=f"rB{qc}")
                    nc.vector.reciprocal(rA[:], O_A[64:65, :])
                    nc.vector.reciprocal(rB[:], O_B[64:65, :])
                    if qc == NQC - 1:
                        # tail: fp32 K=1 PE broadcast (score psum idle by now)
                        bc_ps = ps.tile([128, 1024], F32, tag="s", name="bc_tail")
                        nc.tensor.matmul(bc_ps[0:64, 0:512], ONE[0:1, 0:64],
                                         rA[:], start=True, stop=True)
                        nc.tensor.matmul(bc_ps[64:128, 0:512], ONE[0:1, 0:64],
                                         rB[:], start=True, stop=True)
                        nc.vector.tensor_copy(bcs[:], bc_ps[:, 0:512])
                    else:
                        rAd = dr.tile([1, QC], F32, tag="rad", name=f"rAd{qc}")
                        rBd = dr.tile([1, QC], F32, tag="rbd", name=f"rBd{qc}")
                        nc.sync.dma_start(rAd[:], rA[:])
                        nc.sync.dma_start(rBd[:], rB[:])
                        nc.sync.dma_start(bcs[0:64, :], rAd[:].broadcast_to((64, QC)))
                        nc.sync.dma_start(bcs[64:128, :], rBd[:].broadcast_to((64, QC)))
                    attn = sm.tile([128, QC], BF16, tag="attn", name=f"attn{qc}")
                    nc.vector.tensor_mul(attn[0:64, :], O_A[0:64, :], bcs[0:64, :])
                    nc.vector.tensor_mul(attn[64:128, :], O_B[0:64, :], bcs[64:128, :])
                    for half in range(2):
                        y_ps = ps.tile([128, QC], F32, tag="s", name=f"y_ps{qc}_{half}")
                        nc.tensor.matmul(y_ps[:], WP[:, half * 128:(half + 1) * 128],
                                         attn[:], start=True, stop=True)
                        y_sb = yy.tile([128, QC], F32, tag="y", name=f"y_sb{qc}_{half}")
                        nc.scalar.activation(y_sb[:], y_ps[:], AF.Copy)
                        nc.sync.dma_start(yp.ap()[half * 8 + qc], y_sb[:])

                pending = finish
            for f in pv_queue:
                f()
            if pending is not None:
                pending()

    nc.compile()
    return nc


def _get_nc():
    if "nc" not in _CACHE:
        _CACHE["nc"] = _build()
    return _CACHE["nc"]


def build_in_maps(x, gn_gamma, gn_beta, w_qkv, b_qkv, w_proj):
    import ml_dtypes
    sel_np = np.zeros((128, 4), np.float32)
    for c in range(128):
        sel_np[c, c // 32] = 1.0
    selt_np = sel_np.T.copy()
    gmt_np = np.stack([gn_gamma[0:128], gn_gamma[128:256]], axis=1)
    btt_np = np.stack([gn_beta[0:128], gn_beta[128:256]], axis=1)

    in_maps = []
    for core in range(8):
        b, j = core // 2, core % 2
        r0 = 128 * j
        wsl_np = np.zeros((2, 128, 400), np.float32)
        for cc in range(2):
            cols = slice(cc * 128, (cc + 1) * 128)
            wsl_np[cc, :, 0:128] = w_qkv[r0:r0 + 128, cols].T
            wsl_np[cc, :, 128:256] = w_qkv[256 + r0:256 + r0 + 128, cols].T
            wsl_np[cc, :, 256:320] = w_qkv[512 + r0:512 + r0 + 64, cols].T
            wsl_np[cc, :, 328:392] = w_qkv[512 + r0 + 64:512 + r0 + 128, cols].T
        bqk_np = np.stack([b_qkv[r0:r0 + 128], b_qkv[256 + r0:256 + r0 + 128]],
                          axis=1)
        bv_np = np.zeros((1, 144), np.float32)
        bv_np[0, 0:64] = b_qkv[512 + r0:512 + r0 + 64]
        bv_np[0, 64] = 1.0
        bv_np[0, 72:136] = b_qkv[512 + r0 + 64:512 + r0 + 128]
        bv_np[0, 136] = 1.0
        csts_np = np.concatenate([bqk_np, gmt_np, btt_np, sel_np], axis=1)
        xq = np.ascontiguousarray(
            x[b].reshape(2, 128, 4, 1024).transpose(0, 2, 1, 3)
            .reshape(8, 128, 1024).astype(ml_dtypes.bfloat16))
        in_maps.append({
            "xb": xq,
            "wslb": np.ascontiguousarray(wsl_np.astype(ml_dtypes.bfloat16)),
            "csts": np.ascontiguousarray(csts_np),
            "bv16": np.ascontiguousarray(bv_np.astype(ml_dtypes.bfloat16)),
            "selt": selt_np,
            "wpt": np.ascontiguousarray(
                w_proj[:, r0:r0 + 128].T.astype(ml_dtypes.bfloat16)),
        })

    return in_maps


def kernel(x, gn_gamma, gn_beta, w_qkv, b_qkv, w_proj, b_proj, **_unused):
    x = np.ascontiguousarray(np.asarray(x, dtype=np.float32))
    gn_gamma = np.asarray(gn_gamma, dtype=np.float32)
    gn_beta = np.asarray(gn_beta, dtype=np.float32)
    w_qkv = np.asarray(w_qkv, dtype=np.float32)
    b_qkv = np.asarray(b_qkv, dtype=np.float32)
    w_proj = np.asarray(w_proj, dtype=np.float32)
    b_proj = np.asarray(b_proj, dtype=np.float32)

    nc = _get_nc()
    in_maps = build_in_maps(x, gn_gamma, gn_beta, w_qkv, b_qkv, w_proj)
    res = bass_utils.run_bass_kernel_spmd(nc, in_maps, core_ids=list(range(8)))
    _CACHE["last_result"] = res

    out = np.empty((B, C, N), np.float32)
    for b in range(B):
        ypsum = res.results[2 * b]["yp"] + res.results[2 * b + 1]["yp"]
        ypsum = ypsum.reshape(2, 8, 128, 512).transpose(0, 2, 1, 3).reshape(C, N)
        out[b] = ypsum + x[b].reshape(C, N) + b_proj[:, None]
    return out.reshape(B, C, H, W)


# revision 5
# speedup vs baseline: 1.2503x; 1.2178x over previous
"""Trainium2 Bass kernel for an AttentionBlock (GroupNorm + 4-head self-attention + proj).

Sharding: 8 cores = 4 batches x 2 head-pairs. Core c handles batch c//2, heads
{2j, 2j+1} where j = c%2. Each core: groupnorm of x[b] (duplicated across the
pair of cores), QKV for its 128 feature channels, transposed-score flash
attention (no max subtraction -- scores are ~N(0,1)), partial projection.
Host sums the two partial projections per batch and adds residual + proj bias.

Layout on device: features/keys on partitions, tokens on free dim.
  Q, K: bf16 (128 = 2x64 head dims, 4096 tokens)
  VT:   fp8-e5m2 token-major tiles (128 tokens, [V_A(64)|1|V_B(64)|1]) --
        attention output AND softmax denominator in one pass.
  probs: fp8-e5m2, written per 128-key tile by either
        - ACT: activation(Exp, scale=1/8) with e5m2 output, or
        - DVE: Schraudolph bit-trick: u8 = rne(s*log2(e)/2 + 60) IS the e5m2
          bit pattern of exp(s/8) (linear-interp exp2; sat-at-0 kills the
          negative tail). This splits the softmax stream across two engines.
  PV:   DoubleRow fp8 matmuls: one matmul consumes TWO key tiles (256-deep
        contraction) at 0.5 cycles/row -> 4x fewer PE cycles than bf16.
"""
import sys

sys.path.insert(0, "/opt/trn_rl_repo")

import numpy as np

import concourse.bacc as bacc
import concourse.mybir as mybir
import concourse.tile as tile
from concourse import bass_utils

F32 = mybir.dt.float32
F32R = mybir.dt.float32r
BF16 = mybir.dt.bfloat16
E5 = mybir.dt.float8e5
U8 = mybir.dt.uint8
AF = mybir.ActivationFunctionType
ALU = mybir.AluOpType
AX = mybir.AxisListType
DR = mybir.MatmulPerfMode.DoubleRow

B, C, H, W = 4, 256, 64, 64
N = H * W                  # 4096 tokens
NG = 8                     # groupnorm groups
GSZ = C // NG              # 32 channels per group
NQC = 8                    # query chunks of 512
QC = 512
NKT = 32                   # key tiles of 128
KT = 128
NPR = NKT // 2             # kt pairs
INV_GN = 1.0 / GSZ         # stats are per-partition means
SCALE = 1.0 / 8.0          # hd^-0.5
A_SCH = np.log2(np.e) / 8.0 * 4.0   # schraudolph mult (folds 1/8 score scale)
B_SCH = 60.0                        # e5m2 exponent bias 15 << 2

# exp engine schedule: number of DVE (schraudolph) tiles per qc, out of 32.
# qc 0 is DVE-light (DVE busy with K/VT movers there).
DVE_EXPS = [6, 13, 13, 13, 13, 13, 13, 13]

_CACHE: dict = {}


def _sched_dve(qc, kt):
    n = DVE_EXPS[qc]
    if n <= 0:
        return False
    step = 32.0 / n
    # spread n DVE tiles evenly over the 32 kt slots
    return int(kt // step) != int((kt - 1) // step) if kt > 0 else False


def _build():
    nc = bacc.Bacc("TRN2", target_bir_lowering=False, debug=False,
                   enable_asserts=False)

    xb = nc.dram_tensor("xb", [8, 128, 1024], BF16, kind="ExternalInput")
    wslb = nc.dram_tensor("wslb", [2, 128, 400], BF16, kind="ExternalInput")
    csts = nc.dram_tensor("csts", [128, 10], F32, kind="ExternalInput")
    bv16 = nc.dram_tensor("bv16", [1, 144], BF16, kind="ExternalInput")
    selt = nc.dram_tensor("selt", [4, 128], F32, kind="ExternalInput")
    wpt = nc.dram_tensor("wpt", [128, 256], BF16, kind="ExternalInput")
    yp = nc.dram_tensor("yp", [16, 128, 512], F32, kind="ExternalOutput")

    with tile.TileContext(nc) as tc:
        with (
            tc.tile_pool(name="cst", bufs=1) as cst,
            tc.tile_pool(name="big", bufs=1) as big,
            tc.tile_pool(name="pp", bufs=8) as pp,
            tc.tile_pool(name="sm", bufs=2) as sm,
            tc.tile_pool(name="yy", bufs=4) as yy,
            tc.tile_pool(name="dr", bufs=4, space="DRAM") as dr,
            tc.tile_pool(name="ps", bufs=3, space="PSUM") as ps,
            tc.tile_pool(name="po", bufs=2, space="PSUM") as po,
        ):
            # ---- constants ----
            W0 = cst.tile([128, 400], BF16, tag="w0")
            W1 = cst.tile([128, 400], BF16, tag="w1")
            WP = cst.tile([128, 256], BF16, tag="wp")
            CST = cst.tile([128, 10], F32, tag="cst")
            BQK = CST[:, 0:2]
            GAM = CST[:, 2:4]
            BET = CST[:, 4:6]
            SEL = CST[:, 6:10]
            BV = cst.tile([1, 144], BF16, tag="bv")
            SELT = cst.tile([4, 128], F32, tag="selt")
            ONE = cst.tile([1, 128], F32, tag="one")
            ONEB = cst.tile([1, 128], BF16, tag="oneb")
            EPS = cst.tile([128, 1], F32, tag="eps")

            # ---- load x (chunked, stats via one-pass bn_stats) ----
            NCH = 8
            CH = N // NCH   # 512
            X = [big.tile([128, N], BF16, tag=f"x{cc}", name=f"X{cc}") for cc in range(2)]
            Hb = [big.tile([128, N], BF16, tag=f"hb{cc}", name=f"Hb{cc}") for cc in range(2)]
            BNS = [cst.tile([128, NCH * 6], F32, tag=f"bns{cc}", name=f"BNS{cc}") for cc in range(2)]
            MV = [cst.tile([128, 2], F32, tag=f"mv{cc}", name=f"MV{cc}") for cc in range(2)]
            ST = [cst.tile([128, 2], F32, tag=f"st{cc}", name=f"ST{cc}") for cc in range(2)]
            GS = cst.tile([4, 4], F32, tag="gs")
            gs_ps = po.tile([4, 4], F32, tag="po")
            SX = cst.tile([128, 4], F32, tag="sx")
            SQ = cst.tile([128, 4], F32, tag="sq")
            for i in range(4):
                for cc in range(2):
                    dsl = slice(i * 1024, (i + 1) * 1024)
                    nc.sync.dma_start(X[cc][:, dsl], xb.ap()[cc * 4 + i])
                    for h in range(2):
                        j = 2 * i + h
                        sl = slice(j * CH, (j + 1) * CH)
                        if cc == 1 and j < 4:
                            # first-arriving half-1 slices on ACT (idle early);
                            # scratch into Hb[1] (overwritten later by real Hb)
                            nc.scalar.activation(
                                Hb[1][:, sl], X[1][:, sl], AF.Identity,
                                accum_out=SX[:, j:j + 1])
                            nc.scalar.activation(
                                Hb[1][:, sl], X[1][:, sl], AF.Square,
                                accum_out=SQ[:, j:j + 1])
                        else:
                            nc.vector.bn_stats(BNS[cc][:, 6 * j:6 * j + 6],
                                               X[cc][:, sl])
            # weights & consts (needed later than x)
            nc.vector.memset(EPS[:], 1e-5)
            nc.vector.memset(ONE[:], 1.0)
            nc.vector.memset(ONEB[:], 1.0)
            nc.sync.dma_start(CST[:], csts.ap())
            nc.sync.dma_start(BV[:], bv16.ap())
            nc.sync.dma_start(SELT[:], selt.ap())
            nc.sync.dma_start(W0[:], wslb.ap()[0])
            nc.sync.dma_start(W1[:], wslb.ap()[1])
            nc.sync.dma_start(WP[:], wpt.ap())
            for cc in range(2):
                if cc == 0:
                    nc.vector.bn_aggr(MV[0][:], BNS[0][:])
                    # ST = [mean_p, E[x^2]_p]
                    nc.vector.tensor_mul(ST[0][:, 1:2], MV[0][:, 0:1], MV[0][:, 0:1])
                    nc.vector.tensor_add(ST[0][:, 1:2], ST[0][:, 1:2], MV[0][:, 1:2])
                    nc.vector.tensor_copy(ST[0][:, 0:1], MV[0][:, 0:1])
                else:
                    # merge ACT sums (slices 0-3) with bn stats (slices 4-7)
                    nc.vector.bn_aggr(MV[1][:], BNS[1][:, 24:48])
                    sxs = cst.tile([128, 1], F32, tag="sxs")
                    sqs = cst.tile([128, 1], F32, tag="sqs")
                    nc.vector.reduce_sum(sxs[:], SX[:], axis=AX.X)
                    nc.vector.reduce_sum(sqs[:], SQ[:], axis=AX.X)
                    # mean_p = 0.5*mean_bn + sxs/4096
                    nc.vector.tensor_scalar_mul(ST[1][:, 0:1], MV[1][:, 0:1], 0.5)
                    nc.vector.tensor_scalar_mul(sxs[:], sxs[:], 1.0 / 4096.0)
                    nc.vector.tensor_add(ST[1][:, 0:1], ST[1][:, 0:1], sxs[:])
                    # E2_p = 0.5*(var_bn + mean_bn^2) + sqs/4096
                    nc.vector.tensor_mul(ST[1][:, 1:2], MV[1][:, 0:1], MV[1][:, 0:1])
                    nc.vector.tensor_add(ST[1][:, 1:2], ST[1][:, 1:2], MV[1][:, 1:2])
                    nc.vector.tensor_scalar_mul(ST[1][:, 1:2], ST[1][:, 1:2], 0.5)
                    nc.vector.tensor_scalar_mul(sqs[:], sqs[:], 1.0 / 4096.0)
                    nc.vector.tensor_add(ST[1][:, 1:2], ST[1][:, 1:2], sqs[:])
                nc.tensor.matmul(gs_ps[:, 2 * cc:2 * cc + 2], SEL,
                                 ST[cc][:], start=True, stop=True)
                nc.vector.tensor_copy(GS[:, 2 * cc:2 * cc + 2],
                                      gs_ps[:, 2 * cc:2 * cc + 2])

            # per-channel scale/shift: s = gamma/sqrt(var+eps), t = beta - mean*s
            gn_st = []
            for cc in range(2):
                pc_ps = po.tile([128, 2], F32, tag="po")
                nc.tensor.matmul(pc_ps[:], SELT[:], GS[:, 2 * cc:2 * cc + 2],
                                 start=True, stop=True)
                mean = cst.tile([128, 1], F32, tag=f"mean{cc}")
                var = cst.tile([128, 1], F32, tag=f"var{cc}")
                sd = cst.tile([128, 1], F32, tag=f"sd{cc}")
                s_t = cst.tile([128, 1], F32, tag=f"s{cc}")
                t_t = cst.tile([128, 1], F32, tag=f"t{cc}")
                nc.vector.tensor_scalar_mul(mean[:], pc_ps[:, 0:1], INV_GN)
                nc.vector.tensor_scalar_mul(var[:], pc_ps[:, 1:2], INV_GN)
                # var = E[x^2] - mean^2
                nc.vector.scalar_tensor_tensor(
                    out=sd[:], in0=mean[:], scalar=-1.0, in1=mean[:],
                    op0=ALU.mult, op1=ALU.mult)
                nc.vector.tensor_add(var[:], var[:], sd[:])
                nc.scalar.activation(sd[:], var[:], AF.Sqrt, bias=EPS[:])
                nc.vector.reciprocal(s_t[:], sd[:])
                nc.vector.tensor_mul(s_t[:], s_t[:], GAM[:, cc:cc + 1])
                nc.vector.scalar_tensor_tensor(
                    out=t_t[:], in0=mean[:], scalar=-1.0, in1=s_t[:],
                    op0=ALU.mult, op1=ALU.mult)
                nc.vector.tensor_add(t_t[:], t_t[:], BET[:, cc:cc + 1])
                gn_st.append((s_t, t_t))

            # h = x*s + t (bf16); half 0 on DVE (2x mode), half 1 on ACT/Pool
            for i in range(4):
                sl = slice(i * 1024, (i + 1) * 1024)
                s_t, t_t = gn_st[0]
                nc.vector.tensor_scalar(
                    out=Hb[0][:, sl], in0=X[0][:, sl], scalar1=s_t[:],
                    scalar2=t_t[:], op0=ALU.mult, op1=ALU.add)
                s_t, t_t = gn_st[1]
                nc.gpsimd.tensor_scalar(
                    out=Hb[1][:, sl], in0=X[1][:, sl], scalar1=s_t[:],
                    scalar2=t_t[:], op0=ALU.mult, op1=ALU.add)

            # ---- QKV ----
            Q = big.tile([128, N], BF16, tag="q")
            K = big.tile([128, N], BF16, tag="k")
            VT = big.tile([128, NKT * 144], E5, tag="vt")

            def emit_q_chunk(ch):
                tok = slice(ch * QC, (ch + 1) * QC)
                q_ps = ps.tile([128, QC], F32, tag="s", name=f"q_ps{ch}")
                nc.tensor.matmul(q_ps[:], W0[:, 0:128], Hb[0][:, tok],
                                 start=True, stop=False)
                nc.tensor.matmul(q_ps[:], W1[:, 0:128], Hb[1][:, tok],
                                 start=False, stop=True)
                nc.vector.tensor_scalar(out=Q[:, tok], in0=q_ps[:],
                                        scalar1=BQK[:, 0:1], scalar2=None,
                                        op0=ALU.add)

            def emit_k_chunk(ch):
                tok = slice(ch * QC, (ch + 1) * QC)
                k_ps = ps.tile([128, QC], F32, tag="s", name=f"k_ps{ch}")
                nc.tensor.matmul(k_ps[:], W0[:, 128:256], Hb[0][:, tok],
                                 start=True, stop=False)
                nc.tensor.matmul(k_ps[:], W1[:, 128:256], Hb[1][:, tok],
                                 start=False, stop=True)
                nc.vector.tensor_scalar(out=K[:, tok], in0=k_ps[:],
                                        scalar1=BQK[:, 1:2], scalar2=None,
                                        op0=ALU.add)

            def emit_vt_tile(kt):
                tok = slice(kt * KT, (kt + 1) * KT)
                vt_ps = ps.tile([128, 144], F32, tag="s", name=f"vt_ps{kt}")
                nc.tensor.matmul(vt_ps[:], Hb[0][:, tok], W0[:, 256:400],
                                 start=True, stop=False)
                nc.tensor.matmul(vt_ps[:], Hb[1][:, tok], W1[:, 256:400],
                                 start=False, stop=False)
                # V bias (+ the denominator 1s column) via rank-1 accumulate
                nc.tensor.matmul(vt_ps[:], ONEB[0:1, :], BV[:],
                                 start=False, stop=True)
                nc.vector.tensor_copy(VT[:, kt * 144:(kt + 1) * 144], vt_ps[:])

            emit_q_chunk(0)
            emit_k_chunk(0)

            # ---- attention + projection ----
            pending_a = None
            pending_b = None
            pv_queue = []
            for qc in range(NQC):
                qs = slice(qc * QC, (qc + 1) * QC)
                O_A = po.tile([72, QC], F32, tag="po", name=f"O_A{qc}")
                O_B = po.tile([72, QC], F32, tag="po", name=f"O_B{qc}")
                for pr in range(NPR):
                    P8 = pp.tile([128, 2048], E5, tag="p", name=f"p{qc}_{pr}")
                    for sub in range(2):
                        kt = 2 * pr + sub
                        if qc == 0:
                            if kt % 4 == 2 and kt // 4 < 7:
                                emit_k_chunk(kt // 4 + 1)
                            emit_vt_tile(kt)
                        if pr == 2 and sub == 0 and pending_a is not None:
                            pending_a()
                            pending_a = None
                        if pr == 7 and sub == 0 and pending_b is not None:
                            pending_b()
                            pending_b = None
                        if pr == 8 and sub == 0 and qc < NQC - 1:
                            emit_q_chunk(qc + 1)
                        ks = slice(kt * KT, (kt + 1) * KT)
                        s_ps = ps.tile([128, 1024], F32, tag="s",
                                       name=f"s{qc}_{kt}")
                        nc.tensor.matmul(s_ps[:, 0:512], K[0:64, ks],
                                         Q[0:64, qs], start=True, stop=True)
                        nc.tensor.matmul(s_ps[:, 512:1024], K[64:128, ks],
                                         Q[64:128, qs], start=True, stop=True)
                        if len(pv_queue) == 2:
                            pv_queue.pop(0)()
                        dst = P8[:, sub * 1024:(sub + 1) * 1024]
                        if _sched_dve(qc, kt):
                            nc.vector.tensor_scalar(
                                out=dst.bitcast(U8), in0=s_ps[:],
                                scalar1=A_SCH, scalar2=B_SCH,
                                op0=ALU.mult, op1=ALU.add)
                        else:
                            nc.scalar.activation(dst, s_ps[:], AF.Exp,
                                                 scale=SCALE)

                    def _pv(pr=pr, P8=P8, O_A=O_A, O_B=O_B):
                        vt_ap = VT[:].rearrange("p (t x) -> p t x", t=NKT)[
                            :, 2 * pr:2 * pr + 2, :]
                        p_ap = P8[:].rearrange("p (t x) -> p t x", t=2)
                        nc.tensor.matmul(O_A[:], vt_ap[:, :, 0:72],
                                         p_ap[:, :, 0:512],
                                         start=(pr == 0), stop=(pr == NPR - 1),
                                         perf_mode=DR)
                        nc.tensor.matmul(O_B[:], vt_ap[:, :, 72:144],
                                         p_ap[:, :, 512:1024],
                                         start=(pr == 0), stop=(pr == NPR - 1),
                                         perf_mode=DR)
                    pv_queue.append(_pv)

                # finish phase A: recips + bounce-broadcast DMAs. Phase B
                # (attn-mul/proj/y) runs ~5 pairs later so the DMA latency is
                # hidden and never blocks the in-order DVE exp stream.
                bcs = sm.tile([128, QC], F32, tag="bcs", name=f"bcs{qc}")
                rA = sm.tile([1, QC], F32, tag="ra", name=f"rA{qc}")
                rB = sm.tile([1, QC], F32, tag="rb", name=f"rB{qc}")

                def finish_a(qc=qc, O_A=O_A, O_B=O_B, bcs=bcs, rA=rA, rB=rB):
                    nc.vector.reciprocal(rA[:], O_A[64:65, :])
                    nc.vector.reciprocal(rB[:], O_B[64:65, :])
                    if qc == NQC - 1:
                        # tail: fp32 K=1 PE broadcast (score psum idle by now)
                        bc_ps = ps.tile([128, 1024], F32, tag="s", name="bc_tail")
                        nc.tensor.matmul(bc_ps[0:64, 0:512], ONE[0:1, 0:64],
                                         rA[:], start=True, stop=True)
                        nc.tensor.matmul(bc_ps[64:128, 0:512], ONE[0:1, 0:64],
                                         rB[:], start=True, stop=True)
                        nc.vector.tensor_copy(bcs[:], bc_ps[:, 0:512])
                    else:
                        rAd = dr.tile([1, QC], F32, tag="rad", name=f"rAd{qc}")
                        rBd = dr.tile([1, QC], F32, tag="rbd", name=f"rBd{qc}")
                        nc.sync.dma_start(rAd[:], rA[:])
                        nc.sync.dma_start(rBd[:], rB[:])
                        nc.sync.dma_start(bcs[0:64, :], rAd[:].broadcast_to((64, QC)))
                        nc.sync.dma_start(bcs[64:128, :], rBd[:].broadcast_to((64, QC)))

                def finish_b(qc=qc, O_A=O_A, O_B=O_B, bcs=bcs):
                    attn = sm.tile([128, QC], BF16, tag="attn", name=f"attn{qc}")
                    nc.vector.tensor_mul(attn[0:64, :], O_A[0:64, :], bcs[0:64, :])
                    nc.vector.tensor_mul(attn[64:128, :], O_B[0:64, :], bcs[64:128, :])
                    for half in range(2):
                        y_ps = ps.tile([128, QC], F32, tag="s", name=f"y_ps{qc}_{half}")
                        nc.tensor.matmul(y_ps[:], WP[:, half * 128:(half + 1) * 128],
                                         attn[:], start=True, stop=True)
                        y_sb = yy.tile([128, QC], F32, tag="y", name=f"y_sb{qc}_{half}")
                        nc.vector.tensor_copy(y_sb[:], y_ps[:])
                        nc.sync.dma_start(yp.ap()[half * 8 + qc], y_sb[:])

                pending_a = finish_a
                pending_b = finish_b
            for f in pv_queue:
                f()
            if pending_a is not None:
                pending_a()
            if pending_b is not None:
                pending_b()

    nc.compile()
    return nc


def _get_nc():
    if "nc" not in _CACHE:
        _CACHE["nc"] = _build()
    return _CACHE["nc"]


def build_in_maps(x, gn_gamma, gn_beta, w_qkv, b_qkv, w_proj):
    import ml_dtypes
    sel_np = np.zeros((128, 4), np.float32)
    for c in range(128):
        sel_np[c, c // 32] = 1.0
    selt_np = sel_np.T.copy()
    gmt_np = np.stack([gn_gamma[0:128], gn_gamma[128:256]], axis=1)
    btt_np = np.stack([gn_beta[0:128], gn_beta[128:256]], axis=1)

    in_maps = []
    for core in range(8):
        b, j = core // 2, core % 2
        r0 = 128 * j
        wsl_np = np.zeros((2, 128, 400), np.float32)
        for cc in range(2):
            cols = slice(cc * 128, (cc + 1) * 128)
            wsl_np[cc, :, 0:128] = w_qkv[r0:r0 + 128, cols].T
            wsl_np[cc, :, 128:256] = w_qkv[256 + r0:256 + r0 + 128, cols].T
            wsl_np[cc, :, 256:320] = w_qkv[512 + r0:512 + r0 + 64, cols].T
            wsl_np[cc, :, 328:392] = w_qkv[512 + r0 + 64:512 + r0 + 128, cols].T
        bqk_np = np.stack([b_qkv[r0:r0 + 128], b_qkv[256 + r0:256 + r0 + 128]],
                          axis=1)
        bv_np = np.zeros((1, 144), np.float32)
        bv_np[0, 0:64] = b_qkv[512 + r0:512 + r0 + 64]
        bv_np[0, 64] = 1.0
        bv_np[0, 72:136] = b_qkv[512 + r0 + 64:512 + r0 + 128]
        bv_np[0, 136] = 1.0
        csts_np = np.concatenate([bqk_np, gmt_np, btt_np, sel_np], axis=1)
        xq = np.ascontiguousarray(
            x[b].reshape(2, 128, 4, 1024).transpose(0, 2, 1, 3)
            .reshape(8, 128, 1024).astype(ml_dtypes.bfloat16))
        in_maps.append({
            "xb": xq,
            "wslb": np.ascontiguousarray(wsl_np.astype(ml_dtypes.bfloat16)),
            "csts": np.ascontiguousarray(csts_np),
            "bv16": np.ascontiguousarray(bv_np.astype(ml_dtypes.bfloat16)),
            "selt": selt_np,
            "wpt": np.ascontiguousarray(
                w_proj[:, r0:r0 + 128].T.astype(ml_dtypes.bfloat16)),
        })

    return in_maps


def kernel(x, gn_gamma, gn_beta, w_qkv, b_qkv, w_proj, b_proj, **_unused):
    x = np.ascontiguousarray(np.asarray(x, dtype=np.float32))
    gn_gamma = np.asarray(gn_gamma, dtype=np.float32)
    gn_beta = np.asarray(gn_beta, dtype=np.float32)
    w_qkv = np.asarray(w_qkv, dtype=np.float32)
    b_qkv = np.asarray(b_qkv, dtype=np.float32)
    w_proj = np.asarray(w_proj, dtype=np.float32)
    b_proj = np.asarray(b_proj, dtype=np.float32)

    nc = _get_nc()
    in_maps = build_in_maps(x, gn_gamma, gn_beta, w_qkv, b_qkv, w_proj)
    res = bass_utils.run_bass_kernel_spmd(nc, in_maps, core_ids=list(range(8)))
    _CACHE["last_result"] = res

    out = np.empty((B, C, N), np.float32)
    for b in range(B):
        ypsum = res.results[2 * b]["yp"] + res.results[2 * b + 1]["yp"]
        ypsum = ypsum.reshape(2, 8, 128, 512).transpose(0, 2, 1, 3).reshape(C, N)
        out[b] = ypsum + x[b].reshape(C, N) + b_proj[:, None]
    return out.reshape(B, C, H, W)
